# revision 11
# baseline (speedup 1.0000x reference)
"""Trainium2 Bass kernel for nn_CoordsToNRF.

out[b, p] = atom_nc[b, p] * (627.5095*0.529177) / d2(b, ii[p], jj[p]) / 100
with (ii, jj) = strict-lower-triangle pair indices of 512 atoms,
d2 = squared distance between atoms ii and jj of batch b.

Strategy (pure batch-parallel over 8 NeuronCores, 16 batches/core):
  - Pair rows are grouped into 16 blocks of 32 atom-rows; row i (pairs
    (i, j<i)) is padded to width W_r = 32*(r+1), r = i//32 (6.5% pad).
  - The host packs each core's input/output stream in exactly the device
    tile order (block r, then partition = row-in-block*4 + batch-stripe,
    then free = batch-group*W_r + j), so every main HBM transfer is one
    2-D contiguous DMA per row block.  Host does only reindexing.
  - Per coordinate c, (x_ci - x_cj)^2 is computed diff-first (no
    cancellation):
      c=0,1: ScalarE activation  Square(-1*plane + bias), bias = x_ci
      c=2:   VectorE tensor_scalar subtract, then tensor_tensor square
    where `plane` holds x_c,j for the partition's batch, replicated across
    its 32 row-partitions.
  - d2 = sq0+sq1+sq2 (VectorE + GpSimd adds), r = reciprocal_approx_fast
    (VectorE custom op, ~51 ULP), out = (r * C) * atom_nc fused on GpSimd
    scalar_tensor_tensor.  TensorE/PSUM unused.
"""

import sys

if "/opt/trn_rl_repo" not in sys.path:
    sys.path.insert(0, "/opt/trn_rl_repo")

import numpy as np

from concourse import bacc, mybir
import concourse.tile as tile
from concourse.bass_utils import run_bass_kernel_spmd

# ---------------------------------------------------------------- geometry
B = 128          # total batches
NATOMS = 512
NPAIRS = NATOMS * (NATOMS - 1) // 2   # 130816
NCORES = 8
BPC = B // NCORES                     # 16 batches per core
RG = 32                               # atom-rows per block
NBLK = NATOMS // RG                   # 16 row blocks
BG = 4                                # batch-stripes on partitions
NGRP = BPC // BG                      # 4 batch groups along free dim

NR = [RG * (r + 1) for r in range(NBLK)]          # padded row width per block
OFFR = [0]
for _r in range(NBLK):
    OFFR.append(OFFR[-1] + RG * NR[_r])
PADN = OFFR[-1]                                   # 139264 padded pairs/batch
DEVN = BPC * PADN                                 # flat per-core stream size

CC = float(np.float32(627.5095 * 0.529177 / 100.0))

# local batch index b_local = b_loc * NGRP + bg
#   partition p = ri*BG + b_loc , free f = bg*W_r + j
_ii, _jj = np.tril_indices(NATOMS, -1)
_rb = _ii // RG
_rib = _ii % RG
_bl = np.arange(BPC)
_b_loc = (_bl // NGRP)[:, None]
_bg = (_bl % NGRP)[:, None]
_Wr = np.asarray(NR)[_rb][None, :]
DEVMAP = (
    BPC * np.asarray(OFFR[:-1])[_rb][None, :]
    + (_rib[None, :] * BG + _b_loc) * (NGRP * _Wr)
    + _bg * _Wr
    + _jj[None, :]
).astype(np.int64)                                # [BPC, NPAIRS]

F32 = mybir.dt.float32
ALU = mybir.AluOpType
AF = mybir.ActivationFunctionType

# host-side index maps for the tiny per-partition coordinate columns
# xcol[bg][p, c*NBLK + r] = x_c,(RG*r + ri) of batch b_loc*NGRP+bg,
#   with p = ri*BG + b_loc
_P = np.arange(128)
_XC_RI = _P // BG
_XC_BLOC = _P % BG
_F = np.arange(3 * NBLK)
_XC_C = _F // NBLK
_XC_R = _F % NBLK


# ---------------------------------------------------------------- device IR
def _device_kernel(tc, outdev, coordsT, ncdev, xcolh):
    nc = tc.nc
    WMAX = NGRP * NR[-1]  # 2048

    with (
        tc.tile_pool(name="const", bufs=1) as cpool,
        tc.tile_pool(name="io", bufs=3) as iopool,
        tc.tile_pool(name="sq", bufs=2) as sqpool,
    ):
        planes = []
        xcols = []
        for bg in range(NGRP):
            # batches of this free-group: b_local = b_loc*NGRP + bg
            csl = coordsT.rearrange("(bl g) c a -> g bl c a", g=NGRP)[bg]
            # plane[ri*BG + b_loc, c*512 + j] = x_c,j of that batch
            pl = cpool.tile([128, 3 * NATOMS], F32, tag=f"plane{bg}")
            nc.scalar.dma_start(
                out=pl[0:BG, :], in_=csl.rearrange("b c a -> b (c a)")
            )
            sz = BG
            while sz < 128:
                cp = min(sz, 128 - sz)
                nc.scalar.dma_start(out=pl[sz:sz + cp, :], in_=pl[0:cp, :])
                sz += cp
            planes.append(pl)

            # xcol[ri*BG + b_loc, c*NBLK + r] = x_c,(RG*r+ri) of that batch
            xc = cpool.tile([128, 3 * NBLK], F32, tag=f"xcol{bg}")
            nc.scalar.dma_start(out=xc[:, :], in_=xcolh[bg])
            xcols.append(xc)

        for r in range(NBLK):
            W = NR[r]
            W4 = NGRP * W
            blk = BPC * RG * W                     # elems in this block
            io = iopool.tile([128, WMAX], F32, tag="io")
            src = ncdev[BPC * OFFR[r]:BPC * OFFR[r] + blk].rearrange(
                "(p f) -> p f", p=128
            )
            nc.sync.dma_start(out=io[:, :W4], in_=src)

            sq0 = sqpool.tile([128, WMAX], F32, tag="sq0")
            sq1 = sqpool.tile([128, WMAX], F32, tag="sq1")
            sq2 = sqpool.tile([128, WMAX], F32, tag="sq2")
            dti = sqpool.tile([128, WMAX], F32, tag="dti")

            for bg in range(NGRP):
                sl = slice(bg * W, (bg + 1) * W)
                pl = planes[bg]
                xc = xcols[bg]
                # sq_c = (x_ci - x_cj)^2 = Square(-1*plane + x_ci)
                nc.scalar.activation(
                    out=sq0[:, sl], in_=pl[:, 0:W],
                    func=AF.Square,
                    bias=xc[:, 0 * NBLK + r:0 * NBLK + r + 1], scale=-1.0,
                )
                nc.scalar.activation(
                    out=sq1[:, sl], in_=pl[:, NATOMS:NATOMS + W],
                    func=AF.Square,
                    bias=xc[:, 1 * NBLK + r:1 * NBLK + r + 1], scale=-1.0,
                )
                nc.vector.tensor_scalar_sub(
                    dti[:, sl], pl[:, 2 * NATOMS:2 * NATOMS + W],
                    xc[:, 2 * NBLK + r:2 * NBLK + r + 1],
                )
            # sq2 = d*d/C ; t01 = sq0+sq1 ; d2/C = t01/C + sq2
            # r = C/d2 ; out = r * atom_nc          (C folded via 1/C scales)
            nc.vector.scalar_tensor_tensor(
                out=sq2[:, :W4], in0=dti[:, :W4], scalar=1.0 / CC,
                in1=dti[:, :W4], op0=ALU.mult, op1=ALU.mult,
            )
            nc.gpsimd.tensor_add(sq0[:, :W4], sq0[:, :W4], sq1[:, :W4])
            nc.vector.scalar_tensor_tensor(
                out=sq2[:, :W4], in0=sq0[:, :W4], scalar=1.0 / CC,
                in1=sq2[:, :W4], op0=ALU.mult, op1=ALU.add,
            )
            nc.vector.reciprocal_approx_fast(out=sq1[:, :W4], in_=sq2[:, :W4])
            nc.gpsimd.tensor_mul(io[:, :W4], sq1[:, :W4], io[:, :W4])

            dst = outdev[BPC * OFFR[r]:BPC * OFFR[r] + blk].rearrange(
                "(p f) -> p f", p=128
            )
            nc.sync.dma_start(out=dst, in_=io[:, :W4])


_NC_CACHE = None


def _build():
    global _NC_CACHE
    if _NC_CACHE is not None:
        return _NC_CACHE
    nc = bacc.Bacc("TRN2", target_bir_lowering=False, debug=False)
    coordsT = nc.dram_tensor("coordsT", [BPC, 3, NATOMS], F32,
                             kind="ExternalInput").ap()
    ncdev = nc.dram_tensor("ncdev", [DEVN], F32, kind="ExternalInput").ap()
    xcolh = nc.dram_tensor("xcolh", [NGRP, 128, 3 * NBLK], F32,
                           kind="ExternalInput").ap()
    outdev = nc.dram_tensor("outdev", [DEVN], F32, kind="ExternalOutput").ap()
    with tile.TileContext(nc) as tc:
        _device_kernel(tc, outdev, coordsT, ncdev, xcolh)
    nc.compile()
    _NC_CACHE = nc
    return nc


LAST_RESULTS = None


def _ensure_ntff_hook():
    """Shim antenv.axon_hooks (absent in this image) so trace=True can
    capture NTFF profiles via the axon .so. Only used by test harness."""
    import types
    try:
        from antenv.axon_hooks import get_axon_ntff_profile_hook  # noqa: F401
        return
    except ImportError:
        pass
    try:
        import antenv
        if "/root/.axon_site" not in sys.path:
            sys.path.insert(0, "/root/.axon_site")
        from trn_agent_boot.trn_boot import _ntff_profile_via_ctypes

        mod = types.ModuleType("antenv.axon_hooks")
        _h = [None]
        mod.set_axon_ntff_profile_hook = lambda h: _h.__setitem__(0, h)
        mod.get_axon_ntff_profile_hook = lambda: _h[0]
        sys.modules["antenv.axon_hooks"] = mod
        antenv.axon_hooks = mod
        hook = _ntff_profile_via_ctypes("/opt/axon/libaxon_pjrt.so")
        if hook is not None:
            mod.set_axon_ntff_profile_hook(hook)
    except Exception as e:  # profiling is best-effort
        print("ntff hook shim failed:", e)


def pack_inputs(coords, atom_nc):
    """Host-side layout marshalling (pure reindexing)."""
    coordsT = np.ascontiguousarray(coords.transpose(0, 2, 1))  # [B, 3, 512]
    ncdev = np.zeros((NCORES, DEVN), dtype=np.float32)
    ncdev[:, DEVMAP] = atom_nc.reshape(NCORES, BPC, NPAIRS)
    # xcolh[k, bg, p, f] = coords[k*BPC + (p%BG)*NGRP + bg,
    #                             RG*(f%NBLK) + p//BG, f//NBLK]
    bg_idx = np.arange(NGRP)[:, None, None]
    b_abs = (
        np.arange(NCORES)[:, None, None, None] * BPC
        + (_XC_BLOC[:, None] * NGRP + bg_idx)[None]
    )
    atom_idx = (RG * _XC_R[None, :] + _XC_RI[:, None])[None, None]
    c_idx = _XC_C[None, None, None, :]
    xcolh = np.ascontiguousarray(
        coords[b_abs, atom_idx, c_idx], dtype=np.float32
    )
    return coordsT, ncdev, xcolh


def kernel(coords: np.ndarray, atom_nc: np.ndarray, _trace: bool = False):
    global LAST_RESULTS
    coords = np.ascontiguousarray(np.asarray(coords, dtype=np.float32))
    atom_nc = np.ascontiguousarray(np.asarray(atom_nc, dtype=np.float32))
    assert coords.shape == (B, NATOMS, 3) and atom_nc.shape == (B, NPAIRS)

    if _trace:
        _ensure_ntff_hook()
    nc = _build()
    coordsT, ncdev, xcolh = pack_inputs(coords, atom_nc)

    in_maps = [
        {
            "coordsT": coordsT[k * BPC:(k + 1) * BPC],
            "ncdev": ncdev[k],
            "xcolh": xcolh[k],
        }
        for k in range(NCORES)
    ]
    res = run_bass_kernel_spmd(
        nc, in_maps, core_ids=list(range(NCORES)), trace=_trace
    )
    LAST_RESULTS = res

    out = np.empty((B, NPAIRS), dtype=np.float32)
    for k in range(NCORES):
        out[k * BPC:(k + 1) * BPC] = res.results[k]["outdev"][DEVMAP]
    return out


if __name__ == "__main__":
    rng = np.random.default_rng(0)
    c = rng.standard_normal((B, NATOMS, 3)).astype(np.float32)
    a = rng.random((B, NPAIRS), dtype=np.float32)
    o = kernel(c, a)
    print("ok", o.shape, o.dtype, np.isfinite(o).all())


# revision 15
# speedup vs baseline: 1.1500x; 1.1500x over previous
"""Trainium2 Bass kernel for nn_CoordsToNRF.

out[b, p] = atom_nc[b, p] * (627.5095*0.529177) / d2(b, ii[p], jj[p]) / 100
with (ii, jj) = strict-lower-triangle pair indices of 512 atoms,
d2 = squared distance between atoms ii and jj of batch b.

Strategy (pure batch-parallel over 8 NeuronCores, 16 batches/core):
  - Pair rows are grouped into 16 blocks of 32 atom-rows; row i (pairs
    (i, j<i)) is padded to width W_r = 32*(r+1), r = i//32 (6.5% pad).
  - The host packs each core's input/output stream in exactly the device
    tile order (block r, then partition = row-in-block*4 + batch-stripe,
    then free = batch-group*W_r + j), so every main HBM transfer is one
    2-D contiguous DMA per row block.  Host does only reindexing.
  - Per coordinate c, (x_ci - x_cj)^2 is computed diff-first (no
    cancellation):
      c=0,1: ScalarE activation  Square(-1*plane + bias), bias = x_ci
      c=2:   VectorE tensor_scalar subtract, then tensor_tensor square
    where `plane` holds x_c,j for the partition's batch, replicated across
    its 32 row-partitions.
  - d2 = sq0+sq1+sq2 (VectorE + GpSimd adds), r = reciprocal_approx_fast
    (VectorE custom op, ~51 ULP), out = (r * C) * atom_nc fused on GpSimd
    scalar_tensor_tensor.  TensorE/PSUM unused.
"""

import sys

if "/opt/trn_rl_repo" not in sys.path:
    sys.path.insert(0, "/opt/trn_rl_repo")

import numpy as np

from concourse import bacc, mybir
import concourse.tile as tile
from concourse.bass_utils import run_bass_kernel_spmd
import concourse.dve_ops as dve_ops
from concourse.dve_spec import Spec, Src0, Src1, C0, lower, _has_src1, sq
from concourse.dve_uop import DveOpSpec

# ---------------------------------------------------------------- geometry
B = 128          # total batches
NATOMS = 512
NPAIRS = NATOMS * (NATOMS - 1) // 2   # 130816
NCORES = 8
BPC = B // NCORES                     # 16 batches per core
RG = 32                               # atom-rows per block
NBLK = NATOMS // RG                   # 16 row blocks
BG = 4                                # batch-stripes on partitions
NGRP = BPC // BG                      # 4 batch groups along free dim

NR = [RG * (r + 1) for r in range(NBLK)]          # padded row width per block
OFFR = [0]
for _r in range(NBLK):
    OFFR.append(OFFR[-1] + RG * NR[_r])
PADN = OFFR[-1]                                   # 139264 padded pairs/batch
DEVN = BPC * PADN                                 # flat per-core stream size

CC = float(np.float32(627.5095 * 0.529177 / 100.0))

# local batch index b_local = b_loc * NGRP + bg
#   partition p = ri*BG + b_loc , free f = bg*W_r + j
_ii, _jj = np.tril_indices(NATOMS, -1)
_rb = _ii // RG
_rib = _ii % RG
_bl = np.arange(BPC)
_b_loc = (_bl // NGRP)[:, None]
_bg = (_bl % NGRP)[:, None]
_Wr = np.asarray(NR)[_rb][None, :]
DEVMAP = (
    BPC * np.asarray(OFFR[:-1])[_rb][None, :]
    + (_rib[None, :] * BG + _b_loc) * (NGRP * _Wr)
    + _bg * _Wr
    + _jj[None, :]
).astype(np.int64)                                # [BPC, NPAIRS]

F32 = mybir.dt.float32
ALU = mybir.AluOpType
AF = mybir.ActivationFunctionType

# --- custom DVE op: out = (in0 - in1)^2 * s0  (diff-first, one pass) ------
def _register_sqdiff():
    name = "SQDIFF_SCALED_ANT"
    if name in dve_ops._SUB_OPCODE_FOR_NAME:
        return next(op for op in dve_ops.OPS if op.name == name)
    spec = Spec(
        body=sq(Src0 - Src1) * C0,
        reference=lambda in0, in1, c0, c1, c2: ((in0 - in1) ** 2) * c0,
    )
    row = max(dve_ops._SUB_OPCODE_FOR_NAME.values()) + 1
    shas = {}
    for ver in ("v3", "v4"):
        tmp = DveOpSpec(name=name, opcode=row, uops=lower(spec, ver=ver),
                        rd1_en=_has_src1(spec))
        shas[ver] = tmp.sha(ver)
    op = dve_ops.DveOp(name, spec, subdim=False, uops_sha=shas)
    dve_ops.OPS.append(op)
    dve_ops._SUB_OPCODE_FOR_NAME[name] = row
    dve_ops.CUSTOM_DVE_SPECS[name] = spec
    return op


SQDIFF = _register_sqdiff()

# host-side index maps for the tiny per-partition coordinate columns
# xcol[bg][p, c*NBLK + r] = x_c,(RG*r + ri) of batch b_loc*NGRP+bg,
#   with p = ri*BG + b_loc
_P = np.arange(128)
_XC_RI = _P // BG
_XC_BLOC = _P % BG
_F = np.arange(3 * NBLK)
_XC_C = _F // NBLK
_XC_R = _F % NBLK


# ---------------------------------------------------------------- device IR
def _device_kernel(tc, outdev, coordsT, ncdev, xcolh):
    nc = tc.nc
    WMAX = NGRP * NR[-1]  # 2048

    with (
        tc.tile_pool(name="const", bufs=1) as cpool,
        tc.tile_pool(name="io", bufs=3) as iopool,
        tc.tile_pool(name="sq", bufs=2) as sqpool,
    ):
        planes = []
        xcols = []
        for bg in range(NGRP):
            # batches of this free-group: b_local = b_loc*NGRP + bg
            csl = coordsT.rearrange("(bl g) c a -> g bl c a", g=NGRP)[bg]
            # plane[ri*BG + b_loc, c*512 + j] = x_c,j of that batch
            pl = cpool.tile([128, 3 * NATOMS], F32, tag=f"plane{bg}")
            nc.sync.dma_start(
                out=pl[0:BG, :], in_=csl.rearrange("b c a -> b (c a)")
            )
            sz = BG
            while sz < 128:
                cp = min(sz, 128 - sz)
                nc.sync.dma_start(out=pl[sz:sz + cp, :], in_=pl[0:cp, :])
                sz += cp
            planes.append(pl)

            # xcol[ri*BG + b_loc, c*NBLK + r] = x_c,(RG*r+ri) of that batch
            xc = cpool.tile([128, 3 * NBLK], F32, tag=f"xcol{bg}")
            nc.sync.dma_start(out=xc[:, :], in_=xcolh[bg])
            xcols.append(xc)

        for r in range(NBLK):
            W = NR[r]
            W4 = NGRP * W
            blk = BPC * RG * W                     # elems in this block
            io = iopool.tile([128, WMAX], F32, tag="io")
            src = ncdev[BPC * OFFR[r]:BPC * OFFR[r] + blk].rearrange(
                "(p f) -> p f", p=128
            )
            nc.sync.dma_start(out=io[:, :W4], in_=src)

            sq0 = sqpool.tile([128, WMAX], F32, tag="sq0")
            sq1 = sqpool.tile([128, WMAX], F32, tag="sq1")
            sq2 = sqpool.tile([128, WMAX], F32, tag="sq2")

            for bg in range(NGRP):
                sl = slice(bg * W, (bg + 1) * W)
                pl = planes[bg]
                xc = xcols[bg]
                # sq_c = (x_ci - x_cj)^2 = Square(-1*plane + x_ci)
                nc.scalar.activation(
                    out=sq0[:, sl], in_=pl[:, 0:W],
                    func=AF.Square,
                    bias=xc[:, 0 * NBLK + r:0 * NBLK + r + 1], scale=-1.0,
                )
                nc.scalar.activation(
                    out=sq1[:, sl], in_=pl[:, NATOMS:NATOMS + W],
                    func=AF.Square,
                    bias=xc[:, 1 * NBLK + r:1 * NBLK + r + 1], scale=-1.0,
                )
                # sq2 = (x_2j - x_2i)^2 / C   (custom fused diff-square)
                nc.vector._custom_dve(
                    SQDIFF, out=sq2[:, sl],
                    in0=pl[:, 2 * NATOMS:2 * NATOMS + W],
                    in1=xc[:, 2 * NBLK + r:2 * NBLK + r + 1].to_broadcast(
                        [128, W]
                    ),
                    s0=1.0 / CC,
                )
            # t01 = sq0+sq1 ; d2/C = t01/C + sq2 ; r = C/d2 ; out = r*atom_nc
            nc.gpsimd.tensor_add(sq0[:, :W4], sq0[:, :W4], sq1[:, :W4])
            nc.vector.scalar_tensor_tensor(
                out=sq2[:, :W4], in0=sq0[:, :W4], scalar=1.0 / CC,
                in1=sq2[:, :W4], op0=ALU.mult, op1=ALU.add,
            )
            nc.vector.reciprocal_approx_fast(out=sq1[:, :W4], in_=sq2[:, :W4])
            if r % 2 == 0:
                nc.vector.tensor_mul(io[:, :W4], sq1[:, :W4], io[:, :W4])
            else:
                nc.gpsimd.tensor_mul(io[:, :W4], sq1[:, :W4], io[:, :W4])

            dst = outdev[BPC * OFFR[r]:BPC * OFFR[r] + blk].rearrange(
                "(p f) -> p f", p=128
            )
            nc.sync.dma_start(out=dst, in_=io[:, :W4])


_NC_CACHE = None


def _build():
    global _NC_CACHE
    if _NC_CACHE is not None:
        return _NC_CACHE
    nc = bacc.Bacc("TRN2", target_bir_lowering=False, debug=False)
    coordsT = nc.dram_tensor("coordsT", [BPC, 3, NATOMS], F32,
                             kind="ExternalInput").ap()
    ncdev = nc.dram_tensor("ncdev", [DEVN], F32, kind="ExternalInput").ap()
    xcolh = nc.dram_tensor("xcolh", [NGRP, 128, 3 * NBLK], F32,
                           kind="ExternalInput").ap()
    outdev = nc.dram_tensor("outdev", [DEVN], F32, kind="ExternalOutput").ap()
    with tile.TileContext(nc) as tc:
        _device_kernel(tc, outdev, coordsT, ncdev, xcolh)
    nc.compile()
    _NC_CACHE = nc
    return nc


LAST_RESULTS = None


def _ensure_ntff_hook():
    """Shim antenv.axon_hooks (absent in this image) so trace=True can
    capture NTFF profiles via the axon .so. Only used by test harness."""
    import types
    try:
        from antenv.axon_hooks import get_axon_ntff_profile_hook  # noqa: F401
        return
    except ImportError:
        pass
    try:
        import antenv
        if "/root/.axon_site" not in sys.path:
            sys.path.insert(0, "/root/.axon_site")
        from trn_agent_boot.trn_boot import _ntff_profile_via_ctypes

        mod = types.ModuleType("antenv.axon_hooks")
        _h = [None]
        mod.set_axon_ntff_profile_hook = lambda h: _h.__setitem__(0, h)
        mod.get_axon_ntff_profile_hook = lambda: _h[0]
        sys.modules["antenv.axon_hooks"] = mod
        antenv.axon_hooks = mod
        hook = _ntff_profile_via_ctypes("/opt/axon/libaxon_pjrt.so")
        if hook is not None:
            mod.set_axon_ntff_profile_hook(hook)
    except Exception as e:  # profiling is best-effort
        print("ntff hook shim failed:", e)


def pack_inputs(coords, atom_nc):
    """Host-side layout marshalling (pure reindexing)."""
    coordsT = np.ascontiguousarray(coords.transpose(0, 2, 1))  # [B, 3, 512]
    ncdev = np.zeros((NCORES, DEVN), dtype=np.float32)
    ncdev[:, DEVMAP] = atom_nc.reshape(NCORES, BPC, NPAIRS)
    # xcolh[k, bg, p, f] = coords[k*BPC + (p%BG)*NGRP + bg,
    #                             RG*(f%NBLK) + p//BG, f//NBLK]
    bg_idx = np.arange(NGRP)[:, None, None]
    b_abs = (
        np.arange(NCORES)[:, None, None, None] * BPC
        + (_XC_BLOC[:, None] * NGRP + bg_idx)[None]
    )
    atom_idx = (RG * _XC_R[None, :] + _XC_RI[:, None])[None, None]
    c_idx = _XC_C[None, None, None, :]
    xcolh = np.ascontiguousarray(
        coords[b_abs, atom_idx, c_idx], dtype=np.float32
    )
    return coordsT, ncdev, xcolh


def kernel(coords: np.ndarray, atom_nc: np.ndarray, _trace: bool = False):
    global LAST_RESULTS
    coords = np.ascontiguousarray(np.asarray(coords, dtype=np.float32))
    atom_nc = np.ascontiguousarray(np.asarray(atom_nc, dtype=np.float32))
    assert coords.shape == (B, NATOMS, 3) and atom_nc.shape == (B, NPAIRS)

    if _trace:
        _ensure_ntff_hook()
    nc = _build()
    coordsT, ncdev, xcolh = pack_inputs(coords, atom_nc)

    in_maps = [
        {
            "coordsT": coordsT[k * BPC:(k + 1) * BPC],
            "ncdev": ncdev[k],
            "xcolh": xcolh[k],
        }
        for k in range(NCORES)
    ]
    res = run_bass_kernel_spmd(
        nc, in_maps, core_ids=list(range(NCORES)), trace=_trace
    )
    LAST_RESULTS = res

    out = np.empty((B, NPAIRS), dtype=np.float32)
    for k in range(NCORES):
        out[k * BPC:(k + 1) * BPC] = res.results[k]["outdev"][DEVMAP]
    return out


if __name__ == "__main__":
    rng = np.random.default_rng(0)
    c = rng.standard_normal((B, NATOMS, 3)).astype(np.float32)
    a = rng.random((B, NPAIRS), dtype=np.float32)
    o = kernel(c, a)
    print("ok", o.shape, o.dtype, np.isfinite(o).all())
